# revision 16
# baseline (speedup 1.0000x reference)
import sys

import numpy as np

sys.path.insert(0, "/opt/trn_rl_repo")

import concourse.bacc as bacc
import concourse.mybir as mybir
import concourse.tile as tile
from concourse import bass_utils

B, S, H, NH, D, KS = 2, 2048, 1024, 16, 64, 3
NCORES = 8
GPB = 4
DL = H // GPB
NHL = NH // GPB
P = 128
NPAIR = 2
HC = H // P
SBK = 512
NSB = S // SBK
KT = S // P
f32 = mybir.dt.float32
f32r = mybir.dt.float32r
FT = mybir.ActivationFunctionType
SCALE = float(D) ** -0.5
REPLICA_GROUPS = [[0, 1, 2, 3], [4, 5, 6, 7]]


def build(mask_zero: bool):
    nc = bacc.Bacc("TRN2", target_bir_lowering=False, debug=False, num_devices=NCORES)
    io = dict(
        xqT=nc.dram_tensor("xqT", [H, S + 2], f32, kind="ExternalInput").ap(),
        xsT=nc.dram_tensor("xsT", [H, S + 2], f32, kind="ExternalInput").ap(),
        wq=nc.dram_tensor("wq", [KS, H, DL], f32, kind="ExternalInput").ap(),
        wk=nc.dram_tensor("wk", [KS, H, DL], f32, kind="ExternalInput").ap(),
        wv=nc.dram_tensor("wv", [KS, H, DL], f32, kind="ExternalInput").ap(),
        wo=nc.dram_tensor("wo", [H, DL], f32, kind="ExternalInput").ap(),
        bq=nc.dram_tensor("bq", [NPAIR, P, 1], f32, kind="ExternalInput").ap(),
        bk=nc.dram_tensor("bk", [NPAIR, P, 1], f32, kind="ExternalInput").ap(),
        bv=nc.dram_tensor("bv", [D, NHL], f32, kind="ExternalInput").ap(),
        bo_b=nc.dram_tensor("bo_b", [P, DL], f32, kind="ExternalInput").ap(),
        maskT=nc.dram_tensor("maskT", [P, KT], f32, kind="ExternalInput").ap(),
        ones4=nc.dram_tensor("ones4", [P, NHL], f32, kind="ExternalInput").ap(),
        ones_r=nc.dram_tensor("ones_r", [1, P], f32, kind="ExternalInput").ap(),
        out=nc.dram_tensor("out", [S, DL], f32, kind="ExternalOutput").ap(),
    )
    with tile.TileContext(nc) as tc:
        _body(nc, tc, io, mask_zero)
    nc.finalize()
    return nc


def _body(nc, tc, io, mask_zero):
    xqT, xsT = io["xqT"], io["xsT"]
    wq, wk, wv, wo = io["wq"], io["wk"], io["wv"], io["wo"]
    bq, bk, bv, bo_b, maskT, out = (
        io["bq"], io["bk"], io["bv"], io["bo_b"], io["maskT"], io["out"],
    )
    ones4, ones_r = io["ones4"], io["ones_r"]

    WCOLS = KS * HC * DL

    constp = tc.alloc_tile_pool(name="constp", bufs=1)
    ones_row = constp.tile([D + 1, P], f32r, name="ones_row")
    nc.sync.dma_start(ones_row[D : D + 1, :], ones_r.bitcast(f32r))
    bq_sb = constp.tile([P, NPAIR], f32, name="bq_sb")
    bk_sb = constp.tile([P, NPAIR], f32, name="bk_sb")
    for pr in range(NPAIR):
        nc.sync.dma_start(bq_sb[:, pr : pr + 1], bq[pr])
        nc.sync.dma_start(bk_sb[:, pr : pr + 1], bk[pr])
    bv_sb = constp.tile([D, NHL], f32, name="bv_sb")
    nc.sync.dma_start(bv_sb[:], bv)
    bo_sb = constp.tile([P, DL], f32, name="bo_sb")
    nc.sync.dma_start(bo_sb[:], bo_b)
    maskS = constp.tile([P, KT], f32, name="maskS")
    if not mask_zero:
        mraw = constp.tile([P, KT], f32, name="mraw")
        nc.sync.dma_start(mraw[:], maskT)
        nc.vector.tensor_scalar_mul(maskS[:], mraw[:], -1.0e9)
    wo_sb = constp.tile([P, HC * DL], f32r, name="wo_sb")
    for hc in range(HC):
        nc.sync.dma_start(
            wo_sb[:, hc * DL : (hc + 1) * DL], wo[hc * P : (hc + 1) * P, :].bitcast(f32r)
        )

    bigp = tc.alloc_tile_pool(name="bigp", bufs=1)
    kT = [bigp.tile([P, S], f32r, name=f"kT{pr}") for pr in range(NPAIR)]
    qT = [bigp.tile([P, S], f32r, name=f"qT{pr}") for pr in range(NPAIR)]
    vt = [bigp.tile([P, (D + 1) * NHL], f32r, name=f"vt{st}") for st in range(KT)]
    aTh = [bigp.tile([D, S], f32, name=f"aTh{i}") for i in range(NHL)]

    xwp = tc.alloc_tile_pool(name="xwp", bufs=2)

    wkvp = tc.alloc_tile_pool(name="wkvp", bufs=1)
    wk_sb = wkvp.tile([P, WCOLS], f32r, name="wk_sb")
    wv_sb = wkvp.tile([P, WCOLS], f32r, name="wv_sb")
    for t in range(KS):
        for hc in range(HC):
            cs = (t * HC + hc) * DL
            nc.sync.dma_start(wk_sb[:, cs : cs + DL], wk[t, hc * P : (hc + 1) * P, :].bitcast(f32r))
            nc.sync.dma_start(wv_sb[:, cs : cs + DL], wv[t, hc * P : (hc + 1) * P, :].bitcast(f32r))

    def conv_pair(psum, w_sb, win, pr, idx):
        t, hc = divmod(idx, HC)
        cs = (t * HC + hc) * DL
        lhs = w_sb[:, cs + pr * P : cs + (pr + 1) * P]
        nc.tensor.matmul(
            psum[:], lhs, win[hc][:, t : t + SBK],
            start=(idx == 0), stop=(idx == KS * HC - 1),
        )

    def load_windows(x_dram, sb):
        win = []
        for hc in range(HC):
            w_ = xwp.tile([P, SBK + 2], f32r, name=f"xw{hc}", tag=f"xw{hc}")
            nc.sync.dma_start(
                w_[:], x_dram[hc * P : (hc + 1) * P, sb * SBK : sb * SBK + SBK + 2].bitcast(f32r)
            )
            win.append(w_)
        return win

    with tc.tile_pool(name="psk", bufs=2, space="PSUM") as psk, tc.tile_pool(
        name="psv", bufs=2, space="PSUM"
    ) as psv:
        for sb in range(NSB):
            win = load_windows(xsT, sb)
            for pr in range(NPAIR):
                pk = psk.tile([P, SBK], f32, name="pk", tag="pk")
                for idx in range(KS * HC):
                    conv_pair(pk, wk_sb, win, pr, idx)
                nc.vector.tensor_scalar_add(
                    kT[pr][:, sb * SBK : (sb + 1) * SBK], pk[:], bk_sb[:, pr : pr + 1]
                )
            for j in range(SBK // P):
                st = sb * (SBK // P) + j
                pv = psv.tile([P, DL], f32, name="pv", tag="pv")
                for idx in range(KS * HC):
                    t, hc = divmod(idx, HC)
                    cs = (t * HC + hc) * DL
                    nc.tensor.matmul(
                        pv[:], win[hc][:, j * P + t : j * P + t + P],
                        wv_sb[:, cs : cs + DL],
                        start=(idx == 0), stop=(idx == KS * HC - 1),
                    )
                vt3 = vt[st][:, 0 : NHL * (D + 1)].rearrange("p (h c) -> p h c", c=D + 1)
                nc.vector.tensor_copy(
                    vt3[:, :, 0:D], pv[:].rearrange("p (h c) -> p h c", c=D)
                )
                nc.sync.dma_start(vt3[:, :, D : D + 1], ones4.bitcast(f32r))
    wkvp.release()

    wqp = tc.alloc_tile_pool(name="wqp", bufs=1)
    wq_sb = wqp.tile([P, WCOLS], f32r, name="wq_sb")
    for t in range(KS):
        for hc in range(HC):
            cs = (t * HC + hc) * DL
            nc.sync.dma_start(wq_sb[:, cs : cs + DL], wq[t, hc * P : (hc + 1) * P, :].bitcast(f32r))

    ptp = tc.alloc_tile_pool(name="ptp", bufs=2)
    srecp = tc.alloc_tile_pool(name="srecp", bufs=2)

    ps_s4 = tc.alloc_tile_pool(name="ps_s4", bufs=1, space="PSUM")
    ps_u = tc.alloc_tile_pool(name="ps_u", bufs=1, space="PSUM")
    ps_qe = tc.alloc_tile_pool(name="ps_qe", bufs=1, space="PSUM")
    ps_q = tc.alloc_tile_pool(name="ps_q", bufs=1, space="PSUM")

    def attention(pr, qb):
        qsl = slice(qb * SBK, (qb + 1) * SBK)
        u = [ps_u.tile([D + 1, SBK], f32, name=f"u{h}", tag=f"u{h}") for h in range(2)]
        for kt2 in range(KT // 2):
            kt0, kt1 = 2 * kt2, 2 * kt2 + 1
            s4 = ps_s4.tile([P, 4 * SBK], f32, name="s4", tag="s4")
            for h, (ha, hb) in enumerate(((0, P // 2), (P // 2, P))):
                hsl = slice(ha, hb)
                for i, kt in enumerate((kt0, kt1)):
                    ksl = slice(kt * P, (kt + 1) * P)
                    nc.tensor.matmul(
                        s4[:, (2 * h + i) * SBK : (2 * h + i + 1) * SBK],
                        kT[pr][hsl, ksl],
                        qT[pr][hsl, qsl],
                        start=True, stop=True,
                        tile_position=(ha, 0),
                    )
            pt = ptp.tile([P, 4 * SBK], f32r, name="pt", tag="pt")
            if mask_zero:
                nc.scalar.activation(pt[:], s4[:], FT.Exp, scale=SCALE)
            else:
                for h in range(2):
                    for i, kt in enumerate((kt0, kt1)):
                        csl = slice((2 * h + i) * SBK, (2 * h + i + 1) * SBK)
                        nc.scalar.activation(
                            pt[:, csl], s4[:, csl], FT.Exp,
                            bias=maskS[:, kt : kt + 1], scale=SCALE,
                        )
            for i, kt in enumerate((kt0, kt1)):
                for h in range(2):
                    nc.tensor.matmul(
                        u[h][:, :],
                        vt[kt][:, (2 * pr + h) * (D + 1) : (2 * pr + h + 1) * (D + 1)],
                        pt[:, (2 * h + i) * SBK : (2 * h + i + 1) * SBK],
                        start=(kt2 == 0 and i == 0),
                        stop=(kt2 == KT // 2 - 1 and i == 1),
                    )
        for h in range(2):
            hh = 2 * pr + h
            s_sb = srecp.tile([D + 1, SBK], f32, name="s_sb", tag="s_sb")
            nc.vector.tensor_copy(s_sb[D : D + 1, :], u[h][D : D + 1, :])
            srec = srecp.tile([D + 1, SBK], f32r, name="srec", tag="srec")
            with nc.allow_low_precision(reason="softmax 1/s at fp22 is plenty"):
                nc.vector.reciprocal(srec[D : D + 1, :], s_sb[D : D + 1, :])
            bc = ps_qe.tile([D, SBK], f32, name="bc", tag="qe")
            nc.tensor.matmul(
                bc[:, :], ones_row[D : D + 1, 0:D], srec[D : D + 1, :],
                start=True, stop=True, tile_position=(D, 0),
            )
            bc_sb = srecp.tile([D, SBK], f32, name="bc_sb", tag="bc_sb")
            nc.vector.tensor_copy(bc_sb[:], bc[:])
            nc.vector.tensor_mul(aTh[hh][:, qsl], u[h][0:D, :], bc_sb[:])
            nc.vector.tensor_scalar_add(
                aTh[hh][:, qsl], aTh[hh][:, qsl], bv_sb[:, hh : hh + 1]
            )

    for sb in range(NSB):
        win = load_windows(xqT, sb)
        for pr in range(NPAIR):
            pq = ps_q.tile([P, SBK], f32, name="pq", tag="pq")
            for idx in range(KS * HC):
                conv_pair(pq, wq_sb, win, pr, idx)
            nc.vector.tensor_scalar_add(
                qT[pr][:, sb * SBK : (sb + 1) * SBK], pq[:], bq_sb[:, pr : pr + 1]
            )
        for pr in range(NPAIR):
            attention(pr, sb)

    for pool in (ps_q, ps_qe, ps_u, ps_s4, srecp, ptp, wqp, xwp):
        pool.release()

    dramp = tc.alloc_tile_pool(name="dramp", bufs=1, space="DRAM")
    agin = dramp.tile([DL, S], f32, name="agin")
    agout = dramp.tile([H, S], f32, name="agout")
    for hh in range(NHL):
        nc.sync.dma_start(agin[hh * D : (hh + 1) * D, :], aTh[hh][:])
    nc.gpsimd.collective_compute(
        "AllGather",
        mybir.AluOpType.bypass,
        replica_groups=REPLICA_GROUPS,
        ins=[agin.opt()],
        outs=[agout.opt()],
    )

    outp = tc.alloc_tile_pool(name="outp", bufs=1)
    aTs = [outp.tile([P, S], f32r, name=f"aTs{hc}") for hc in range(HC)]
    for hc in range(HC):
        nc.sync.dma_start(aTs[hc][:], agout[hc * P : (hc + 1) * P, :].bitcast(f32r))
    with tc.tile_pool(name="osb_p", bufs=3) as osb_p, tc.tile_pool(
        name="ps_o", bufs=8, space="PSUM"
    ) as ps_o:
        for qt in range(S // P):
            po = ps_o.tile([P, DL], f32, name="po", tag="po")
            for hc in range(HC):
                nc.tensor.matmul(
                    po[:],
                    aTs[hc][:, qt * P : (qt + 1) * P],
                    wo_sb[:, hc * DL : (hc + 1) * DL],
                    start=(hc == 0), stop=(hc == HC - 1),
                )
            osb = osb_p.tile([P, DL], f32, name="osb", tag="osb")
            nc.vector.tensor_add(osb[:], po[:], bo_sb[:])
            nc.sync.dma_start(out[qt * P : (qt + 1) * P, :], osb[:])
    outp.release()
    dramp.release()
    bigp.release()
    constp.release()


_BUILD_CACHE = {}


def get_build(mask_zero: bool):
    if mask_zero not in _BUILD_CACHE:
        _BUILD_CACHE[mask_zero] = build(mask_zero)
    return _BUILD_CACHE[mask_zero]


def shard_inputs(inputs):
    qi = np.ascontiguousarray(np.asarray(inputs["query_input"], dtype=np.float32))
    si = np.ascontiguousarray(np.asarray(inputs["source_input"], dtype=np.float32))
    mask = np.asarray(inputs["mask"], dtype=np.float32)
    Wq = np.asarray(inputs["Wq"], dtype=np.float32)
    Wk = np.asarray(inputs["Wk"], dtype=np.float32)
    Wv = np.asarray(inputs["Wv"], dtype=np.float32)
    Wo = np.asarray(inputs["Wo"], dtype=np.float32)
    bq_ = np.asarray(inputs["bq"], dtype=np.float32)
    bk_ = np.asarray(inputs["bk"], dtype=np.float32)
    bv_ = np.asarray(inputs["bv"], dtype=np.float32)
    bo_ = np.asarray(inputs["bo"], dtype=np.float32)

    xT = []
    for b in range(B):
        for x in (qi, si):
            p = np.zeros((H, S + 2), dtype=np.float32)
            p[:, 1 : S + 1] = x[b].T
            xT.append(np.ascontiguousarray(p))
    in_maps = []
    for c in range(NCORES):
        b, g = divmod(c, GPB)
        cols = slice(g * DL, (g + 1) * DL)
        in_maps.append(
            dict(
                xqT=xT[2 * b],
                xsT=xT[2 * b + 1],
                wq=np.ascontiguousarray(Wq[:, :, cols]),
                wk=np.ascontiguousarray(Wk[:, :, cols]),
                wv=np.ascontiguousarray(Wv[:, :, cols]),
                wo=np.ascontiguousarray(Wo[:, cols]),
                bq=np.ascontiguousarray(bq_[cols].reshape(NPAIR, P, 1)),
                bk=np.ascontiguousarray(bk_[cols].reshape(NPAIR, P, 1)),
                bv=np.ascontiguousarray(bv_[cols].reshape(NHL, D).T),
                bo_b=np.ascontiguousarray(
                    np.broadcast_to(bo_[cols], (P, DL)).astype(np.float32)
                ),
                maskT=np.ascontiguousarray(mask[b, 0, 0].reshape(KT, P).T),
                ones4=np.ones((P, NHL), dtype=np.float32),
                ones_r=np.ones((1, P), dtype=np.float32),
            )
        )
    return in_maps, bool(not np.any(mask))


def unshard(core_outs):
    full = np.empty((B, S, H), dtype=np.float32)
    for c in range(NCORES):
        b, g = divmod(c, GPB)
        full[b, :, g * DL : (g + 1) * DL] = core_outs[c]["out"]
    return full


def run(inputs, **kwargs):
    in_maps, mask_zero = shard_inputs(inputs)
    nc = get_build(mask_zero)
    res = bass_utils.run_bass_kernel_spmd(
        nc, in_maps, core_ids=list(range(NCORES)), **kwargs
    )
    return unshard(res.results), res


def kernel(**inputs):
    out, _ = run(inputs)
    return out


# revision 18
# speedup vs baseline: 1.1903x; 1.1903x over previous
import sys

import ml_dtypes
import numpy as np

sys.path.insert(0, "/opt/trn_rl_repo")

import concourse.bacc as bacc
import concourse.mybir as mybir
import concourse.tile as tile
from concourse import bass_utils

B, S, H, NH, D, KS = 2, 2048, 1024, 16, 64, 3
NCORES = 8
GPB = 4
DL = H // GPB
NHL = NH // GPB
P = 128
NPAIR = 2
HC = H // P
SBK = 512
NSB = S // SBK
KT = S // P
f32 = mybir.dt.float32
f32r = mybir.dt.float32r
bf16 = mybir.dt.bfloat16
FT = mybir.ActivationFunctionType
SCALE = float(D) ** -0.5
REPLICA_GROUPS = [[0, 1, 2, 3], [4, 5, 6, 7]]


def build(mask_zero: bool):
    nc = bacc.Bacc("TRN2", target_bir_lowering=False, debug=False, num_devices=NCORES)
    io = dict(
        xqT=nc.dram_tensor("xqT", [H, S + 2], f32, kind="ExternalInput").ap(),
        xsT=nc.dram_tensor("xsT", [H, S + 2], f32, kind="ExternalInput").ap(),
        wq=nc.dram_tensor("wq", [KS, H, DL], f32, kind="ExternalInput").ap(),
        wk=nc.dram_tensor("wk", [KS, H, DL], f32, kind="ExternalInput").ap(),
        wv=nc.dram_tensor("wv", [KS, H, DL], f32, kind="ExternalInput").ap(),
        wo=nc.dram_tensor("wo", [H, DL], bf16, kind="ExternalInput").ap(),
        bq=nc.dram_tensor("bq", [NPAIR, P, 1], f32, kind="ExternalInput").ap(),
        bk=nc.dram_tensor("bk", [NPAIR, P, 1], f32, kind="ExternalInput").ap(),
        bv=nc.dram_tensor("bv", [D, NHL], f32, kind="ExternalInput").ap(),
        bo_b=nc.dram_tensor("bo_b", [P, DL], f32, kind="ExternalInput").ap(),
        maskT=nc.dram_tensor("maskT", [P, KT], f32, kind="ExternalInput").ap(),
        ones4=nc.dram_tensor("ones4", [P, NHL], f32, kind="ExternalInput").ap(),
        ones_r=nc.dram_tensor("ones_r", [1, P], f32, kind="ExternalInput").ap(),
        out=nc.dram_tensor("out", [S, DL], f32, kind="ExternalOutput").ap(),
    )
    with tile.TileContext(nc) as tc:
        _body(nc, tc, io, mask_zero)
    nc.finalize()
    return nc


def _body(nc, tc, io, mask_zero):
    xqT, xsT = io["xqT"], io["xsT"]
    wq, wk, wv, wo = io["wq"], io["wk"], io["wv"], io["wo"]
    bq, bk, bv, bo_b, maskT, out = (
        io["bq"], io["bk"], io["bv"], io["bo_b"], io["maskT"], io["out"],
    )
    ones4, ones_r = io["ones4"], io["ones_r"]

    WCOLS = KS * HC * DL

    constp = tc.alloc_tile_pool(name="constp", bufs=1)
    ones_row = constp.tile([D + 1, P], f32r, name="ones_row")
    nc.sync.dma_start(ones_row[D : D + 1, :], ones_r.bitcast(f32r))
    bq_sb = constp.tile([P, NPAIR], f32, name="bq_sb")
    bk_sb = constp.tile([P, NPAIR], f32, name="bk_sb")
    for pr in range(NPAIR):
        nc.sync.dma_start(bq_sb[:, pr : pr + 1], bq[pr])
        nc.sync.dma_start(bk_sb[:, pr : pr + 1], bk[pr])
    bv_sb = constp.tile([D, NHL], f32, name="bv_sb")
    nc.sync.dma_start(bv_sb[:], bv)
    bo_sb = constp.tile([P, DL], f32, name="bo_sb")
    nc.sync.dma_start(bo_sb[:], bo_b)
    maskS = constp.tile([P, KT], f32, name="maskS")
    if not mask_zero:
        mraw = constp.tile([P, KT], f32, name="mraw")
        nc.sync.dma_start(mraw[:], maskT)
        nc.vector.tensor_scalar_mul(maskS[:], mraw[:], -1.0e9)
    wo_sb = constp.tile([P, HC * DL], bf16, name="wo_sb")
    for hc in range(HC):
        nc.sync.dma_start(wo_sb[:, hc * DL : (hc + 1) * DL], wo[hc * P : (hc + 1) * P, :])

    bigp = tc.alloc_tile_pool(name="bigp", bufs=1)
    kT = [bigp.tile([P, S], f32r, name=f"kT{pr}") for pr in range(NPAIR)]
    qT = [bigp.tile([P, S], f32r, name=f"qT{pr}") for pr in range(NPAIR)]
    vt = [bigp.tile([P, (D + 1) * NHL], f32r, name=f"vt{st}") for st in range(KT)]
    aTh = [bigp.tile([D, S], bf16, name=f"aTh{i}") for i in range(NHL)]

    xwp = tc.alloc_tile_pool(name="xwp", bufs=2)

    wkvp = tc.alloc_tile_pool(name="wkvp", bufs=1)
    wk_sb = wkvp.tile([P, WCOLS], f32r, name="wk_sb")
    wv_sb = wkvp.tile([P, WCOLS], f32r, name="wv_sb")
    for t in range(KS):
        for hc in range(HC):
            cs = (t * HC + hc) * DL
            nc.sync.dma_start(wk_sb[:, cs : cs + DL], wk[t, hc * P : (hc + 1) * P, :].bitcast(f32r))
            nc.sync.dma_start(wv_sb[:, cs : cs + DL], wv[t, hc * P : (hc + 1) * P, :].bitcast(f32r))

    def conv_pair(psum, w_sb, win, pr, idx):
        t, hc = divmod(idx, HC)
        cs = (t * HC + hc) * DL
        lhs = w_sb[:, cs + pr * P : cs + (pr + 1) * P]
        nc.tensor.matmul(
            psum[:], lhs, win[hc][:, t : t + SBK],
            start=(idx == 0), stop=(idx == KS * HC - 1),
        )

    def load_windows(x_dram, sb):
        win = []
        for hc in range(HC):
            w_ = xwp.tile([P, SBK + 2], f32r, name=f"xw{hc}", tag=f"xw{hc}")
            nc.sync.dma_start(
                w_[:], x_dram[hc * P : (hc + 1) * P, sb * SBK : sb * SBK + SBK + 2].bitcast(f32r)
            )
            win.append(w_)
        return win

    with tc.tile_pool(name="psk", bufs=2, space="PSUM") as psk, tc.tile_pool(
        name="psv", bufs=2, space="PSUM"
    ) as psv:
        for sb in range(NSB):
            win = load_windows(xsT, sb)
            for pr in range(NPAIR):
                pk = psk.tile([P, SBK], f32, name="pk", tag="pk")
                for idx in range(KS * HC):
                    conv_pair(pk, wk_sb, win, pr, idx)
                nc.vector.tensor_scalar_add(
                    kT[pr][:, sb * SBK : (sb + 1) * SBK], pk[:], bk_sb[:, pr : pr + 1]
                )
            for j in range(SBK // P):
                st = sb * (SBK // P) + j
                pv = psv.tile([P, DL], f32, name="pv", tag="pv")
                for idx in range(KS * HC):
                    t, hc = divmod(idx, HC)
                    cs = (t * HC + hc) * DL
                    nc.tensor.matmul(
                        pv[:], win[hc][:, j * P + t : j * P + t + P],
                        wv_sb[:, cs : cs + DL],
                        start=(idx == 0), stop=(idx == KS * HC - 1),
                    )
                vt3 = vt[st][:, 0 : NHL * (D + 1)].rearrange("p (h c) -> p h c", c=D + 1)
                nc.vector.tensor_copy(
                    vt3[:, :, 0:D], pv[:].rearrange("p (h c) -> p h c", c=D)
                )
                nc.sync.dma_start(vt3[:, :, D : D + 1], ones4.bitcast(f32r))
    wkvp.release()

    wqp = tc.alloc_tile_pool(name="wqp", bufs=1)
    wq_sb = wqp.tile([P, WCOLS], f32r, name="wq_sb")
    for t in range(KS):
        for hc in range(HC):
            cs = (t * HC + hc) * DL
            nc.sync.dma_start(wq_sb[:, cs : cs + DL], wq[t, hc * P : (hc + 1) * P, :].bitcast(f32r))

    ptp = tc.alloc_tile_pool(name="ptp", bufs=4)
    srecp = tc.alloc_tile_pool(name="srecp", bufs=2)

    ps_q = tc.alloc_tile_pool(name="ps_q", bufs=1, space="PSUM")
    ps_s4 = tc.alloc_tile_pool(name="ps_s4", bufs=2, space="PSUM")
    ps_u = tc.alloc_tile_pool(name="ps_u", bufs=1, space="PSUM")
    ps_qe = tc.alloc_tile_pool(name="ps_qe", bufs=1, space="PSUM")

    dramp = tc.alloc_tile_pool(name="dramp", bufs=1, space="DRAM")
    agin = [dramp.tile([P, S], bf16, name=f"agin{pr}") for pr in range(NPAIR)]
    agout = [dramp.tile([GPB * P, S], bf16, name=f"agout{pr}") for pr in range(NPAIR)]

    for sb in range(NSB):
        win = load_windows(xqT, sb)
        for pr in range(NPAIR):
            pq = ps_q.tile([P, SBK], f32, name="pq", tag="pq")
            for idx in range(KS * HC):
                conv_pair(pq, wq_sb, win, pr, idx)
            nc.vector.tensor_scalar_add(
                qT[pr][:, sb * SBK : (sb + 1) * SBK], pq[:], bq_sb[:, pr : pr + 1]
            )

    def attention(pr, qb):
        qsl = slice(qb * SBK, (qb + 1) * SBK)
        u = [ps_u.tile([D + 1, SBK], f32, name=f"u{h}", tag=f"u{h}") for h in range(2)]
        for kt2 in range(KT // 2):
            kt0, kt1 = 2 * kt2, 2 * kt2 + 1
            s2 = [ps_s4.tile([P, 2 * SBK], f32, name=f"s2{h}", tag="s4") for h in range(2)]
            pt2 = [ptp.tile([P, 2 * SBK], f32r, name=f"pt{h}", tag="pt") for h in range(2)]
            for h, (ha, hb) in enumerate(((0, P // 2), (P // 2, P))):
                hsl = slice(ha, hb)
                for i, kt in enumerate((kt0, kt1)):
                    ksl = slice(kt * P, (kt + 1) * P)
                    nc.tensor.matmul(
                        s2[h][:, i * SBK : (i + 1) * SBK],
                        kT[pr][hsl, ksl],
                        qT[pr][hsl, qsl],
                        start=True, stop=True,
                        tile_position=(ha, 0),
                    )
                if mask_zero:
                    nc.scalar.activation(pt2[h][:], s2[h][:], FT.Exp, scale=SCALE)
                else:
                    for i, kt in enumerate((kt0, kt1)):
                        csl = slice(i * SBK, (i + 1) * SBK)
                        nc.scalar.activation(
                            pt2[h][:, csl], s2[h][:, csl], FT.Exp,
                            bias=maskS[:, kt : kt + 1], scale=SCALE,
                        )
            for i, kt in enumerate((kt0, kt1)):
                for h in range(2):
                    nc.tensor.matmul(
                        u[h][:, :],
                        vt[kt][:, (2 * pr + h) * (D + 1) : (2 * pr + h + 1) * (D + 1)],
                        pt2[h][:, i * SBK : (i + 1) * SBK],
                        start=(kt2 == 0 and i == 0),
                        stop=(kt2 == KT // 2 - 1 and i == 1),
                    )
        for h in range(2):
            hh = 2 * pr + h
            s_sb = srecp.tile([D + 1, SBK], f32, name="s_sb", tag="s_sb")
            nc.vector.tensor_copy(s_sb[D : D + 1, :], u[h][D : D + 1, :])
            srec = srecp.tile([D + 1, SBK], f32r, name="srec", tag="srec")
            with nc.allow_low_precision(reason="softmax 1/s at fp22 is plenty"):
                nc.vector.reciprocal(srec[D : D + 1, :], s_sb[D : D + 1, :])
            bc = ps_qe.tile([D, SBK], f32, name="bc", tag="qe")
            nc.tensor.matmul(
                bc[:, :], ones_row[D : D + 1, 0:D], srec[D : D + 1, :],
                start=True, stop=True, tile_position=(D, 0),
            )
            bc_sb = srecp.tile([D, SBK], f32, name="bc_sb", tag="bc_sb")
            nc.vector.tensor_copy(bc_sb[:], bc[:])
            nc.vector.tensor_mul(aTh[hh][:, qsl], u[h][0:D, :], bc_sb[:])
            nc.vector.tensor_scalar_add(
                aTh[hh][:, qsl], aTh[hh][:, qsl], bv_sb[:, hh : hh + 1]
            )
            nc.sync.dma_start(
                agin[pr][h * D : (h + 1) * D, qsl], aTh[hh][:, qsl].bitcast(bf16)
            )

    for pr in range(NPAIR):
        for qb in range(NSB):
            attention(pr, qb)
        nc.gpsimd.collective_compute(
            "AllGather",
            mybir.AluOpType.bypass,
            replica_groups=REPLICA_GROUPS,
            ins=[agin[pr].opt()],
            outs=[agout[pr].opt()],
        )

    for pool in (ps_qe, ps_u, ps_s4, ps_q, srecp, ptp, wqp, xwp):
        pool.release()

    outp = tc.alloc_tile_pool(name="outp", bufs=1)
    aTs = []
    for hc in range(HC):
        a_ = outp.tile([P, S], bf16, name=f"aTs{hc}")
        pr, r = divmod(hc, GPB)
        nc.sync.dma_start(a_[:], agout[pr][r * P : (r + 1) * P, :])
        aTs.append(a_)
    with tc.tile_pool(name="osb_p", bufs=3) as osb_p, tc.tile_pool(
        name="ps_o", bufs=8, space="PSUM"
    ) as ps_o:
        for qt in range(S // P):
            po = ps_o.tile([P, DL], f32, name="po", tag="po")
            for hc in range(HC):
                nc.tensor.matmul(
                    po[:],
                    aTs[hc][:, qt * P : (qt + 1) * P],
                    wo_sb[:, hc * DL : (hc + 1) * DL],
                    start=(hc == 0), stop=(hc == HC - 1),
                )
            osb = osb_p.tile([P, DL], f32, name="osb", tag="osb")
            nc.vector.tensor_add(osb[:], po[:], bo_sb[:])
            nc.sync.dma_start(out[qt * P : (qt + 1) * P, :], osb[:])
    outp.release()
    dramp.release()
    bigp.release()
    constp.release()


_BUILD_CACHE = {}


def get_build(mask_zero: bool):
    if mask_zero not in _BUILD_CACHE:
        _BUILD_CACHE[mask_zero] = build(mask_zero)
    return _BUILD_CACHE[mask_zero]


def shard_inputs(inputs):
    qi = np.ascontiguousarray(np.asarray(inputs["query_input"], dtype=np.float32))
    si = np.ascontiguousarray(np.asarray(inputs["source_input"], dtype=np.float32))
    mask = np.asarray(inputs["mask"], dtype=np.float32)
    Wq = np.asarray(inputs["Wq"], dtype=np.float32)
    Wk = np.asarray(inputs["Wk"], dtype=np.float32)
    Wv = np.asarray(inputs["Wv"], dtype=np.float32)
    Wo = np.asarray(inputs["Wo"], dtype=np.float32)
    bq_ = np.asarray(inputs["bq"], dtype=np.float32)
    bk_ = np.asarray(inputs["bk"], dtype=np.float32)
    bv_ = np.asarray(inputs["bv"], dtype=np.float32)
    bo_ = np.asarray(inputs["bo"], dtype=np.float32)

    xT = []
    for b in range(B):
        for x in (qi, si):
            p = np.zeros((H, S + 2), dtype=np.float32)
            p[:, 1 : S + 1] = x[b].T
            xT.append(np.ascontiguousarray(p))
    in_maps = []
    for c in range(NCORES):
        b, g = divmod(c, GPB)
        cols = slice(g * DL, (g + 1) * DL)
        in_maps.append(
            dict(
                xqT=xT[2 * b],
                xsT=xT[2 * b + 1],
                wq=np.ascontiguousarray(Wq[:, :, cols]),
                wk=np.ascontiguousarray(Wk[:, :, cols]),
                wv=np.ascontiguousarray(Wv[:, :, cols]),
                wo=np.ascontiguousarray(
                    np.concatenate(
                        [Wo[r * DL + pr * P : r * DL + (pr + 1) * P] for pr in range(NPAIR) for r in range(GPB)],
                        axis=0,
                    )[:, cols].astype(ml_dtypes.bfloat16)
                ),
                bq=np.ascontiguousarray(bq_[cols].reshape(NPAIR, P, 1)),
                bk=np.ascontiguousarray(bk_[cols].reshape(NPAIR, P, 1)),
                bv=np.ascontiguousarray(bv_[cols].reshape(NHL, D).T),
                bo_b=np.ascontiguousarray(
                    np.broadcast_to(bo_[cols], (P, DL)).astype(np.float32)
                ),
                maskT=np.ascontiguousarray(mask[b, 0, 0].reshape(KT, P).T),
                ones4=np.ones((P, NHL), dtype=np.float32),
                ones_r=np.ones((1, P), dtype=np.float32),
            )
        )
    return in_maps, bool(not np.any(mask))


def unshard(core_outs):
    full = np.empty((B, S, H), dtype=np.float32)
    for c in range(NCORES):
        b, g = divmod(c, GPB)
        full[b, :, g * DL : (g + 1) * DL] = core_outs[c]["out"]
    return full


def run(inputs, **kwargs):
    in_maps, mask_zero = shard_inputs(inputs)
    nc = get_build(mask_zero)
    res = bass_utils.run_bass_kernel_spmd(
        nc, in_maps, core_ids=list(range(NCORES)), **kwargs
    )
    return unshard(res.results), res


def kernel(**inputs):
    out, _ = run(inputs)
    return out


# revision 20
# speedup vs baseline: 1.2510x; 1.0510x over previous
import sys

import ml_dtypes
import numpy as np

sys.path.insert(0, "/opt/trn_rl_repo")

import concourse.bacc as bacc
import concourse.mybir as mybir
import concourse.tile as tile
from concourse import bass_utils

B, S, H, NH, D, KS = 2, 2048, 1024, 16, 64, 3
NCORES = 8
GPB = 4
DL = H // GPB
NHL = NH // GPB
P = 128
NPAIR = 2
HC = H // P
SBK = 512
NSB = S // SBK
KT = S // P
f32 = mybir.dt.float32
f32r = mybir.dt.float32r
bf16 = mybir.dt.bfloat16
FT = mybir.ActivationFunctionType
SCALE = float(D) ** -0.5
REPLICA_GROUPS = [[0, 1, 2, 3], [4, 5, 6, 7]]


def build(mask_zero: bool):
    nc = bacc.Bacc("TRN2", target_bir_lowering=False, debug=False, num_devices=NCORES)
    io = dict(
        xqT=nc.dram_tensor("xqT", [H, S + 2], bf16, kind="ExternalInput").ap(),
        xsT=nc.dram_tensor("xsT", [H, S + 2], bf16, kind="ExternalInput").ap(),
        wq=nc.dram_tensor("wq", [KS, H, DL], bf16, kind="ExternalInput").ap(),
        wk=nc.dram_tensor("wk", [KS, H, DL], bf16, kind="ExternalInput").ap(),
        wv=nc.dram_tensor("wv", [KS, H, DL], bf16, kind="ExternalInput").ap(),
        wo=nc.dram_tensor("wo", [H, DL], bf16, kind="ExternalInput").ap(),
        bq=nc.dram_tensor("bq", [NPAIR, P, 1], f32, kind="ExternalInput").ap(),
        bk=nc.dram_tensor("bk", [NPAIR, P, 1], f32, kind="ExternalInput").ap(),
        bv=nc.dram_tensor("bv", [D, NHL], f32, kind="ExternalInput").ap(),
        bo_b=nc.dram_tensor("bo_b", [P, DL], f32, kind="ExternalInput").ap(),
        maskT=nc.dram_tensor("maskT", [P, KT], f32, kind="ExternalInput").ap(),
        ones4=nc.dram_tensor("ones4", [P, NHL], bf16, kind="ExternalInput").ap(),
        ones_r=nc.dram_tensor("ones_r", [1, P], f32, kind="ExternalInput").ap(),
        out=nc.dram_tensor("out", [S, DL], f32, kind="ExternalOutput").ap(),
    )
    with tile.TileContext(nc) as tc:
        _body(nc, tc, io, mask_zero)
    nc.finalize()
    return nc


def _body(nc, tc, io, mask_zero):
    xqT, xsT = io["xqT"], io["xsT"]
    wq, wk, wv, wo = io["wq"], io["wk"], io["wv"], io["wo"]
    bq, bk, bv, bo_b, maskT, out = (
        io["bq"], io["bk"], io["bv"], io["bo_b"], io["maskT"], io["out"],
    )
    ones4, ones_r = io["ones4"], io["ones_r"]

    constp = tc.alloc_tile_pool(name="constp", bufs=1)
    ones_row = constp.tile([D + 1, P], f32r, name="ones_row")
    nc.sync.dma_start(ones_row[D : D + 1, :], ones_r.bitcast(f32r))
    bq_sb = constp.tile([P, NPAIR], f32, name="bq_sb")
    bk_sb = constp.tile([P, NPAIR], f32, name="bk_sb")
    for pr in range(NPAIR):
        nc.sync.dma_start(bq_sb[:, pr : pr + 1], bq[pr])
        nc.sync.dma_start(bk_sb[:, pr : pr + 1], bk[pr])
    bv_sb = constp.tile([D, NHL], f32, name="bv_sb")
    nc.sync.dma_start(bv_sb[:], bv)
    bo_sb = constp.tile([P, DL], f32, name="bo_sb")
    nc.sync.dma_start(bo_sb[:], bo_b)
    maskS = constp.tile([P, KT], f32, name="maskS")
    if not mask_zero:
        mraw = constp.tile([P, KT], f32, name="mraw")
        nc.sync.dma_start(mraw[:], maskT)
        nc.vector.tensor_scalar_mul(maskS[:], mraw[:], -1.0e9)
    wo_sb = constp.tile([P, HC * DL], bf16, name="wo_sb")
    for hc in range(HC):
        nc.sync.dma_start(wo_sb[:, hc * DL : (hc + 1) * DL], wo[hc * P : (hc + 1) * P, :])

    bigp = tc.alloc_tile_pool(name="bigp", bufs=1)
    kT = [bigp.tile([P, S], bf16, name=f"kT{pr}") for pr in range(NPAIR)]
    qT = [bigp.tile([P, S], bf16, name=f"qT{pr}") for pr in range(NPAIR)]
    vt = [bigp.tile([P, (D + 1) * NHL], bf16, name=f"vt{st}") for st in range(KT)]
    aTh = [bigp.tile([D, S], bf16, name=f"aTh{i}") for i in range(NHL)]

    xwp = tc.alloc_tile_pool(name="xwp", bufs=2)

    def load_w(pool, w_dram, prefix):
        tiles = []
        for t in range(KS):
            for hc in range(HC):
                w_ = pool.tile(
                    [P, DL], bf16, name=f"{prefix}{t}_{hc}", tag=f"{prefix}{t}_{hc}"
                )
                nc.sync.dma_start(w_[:], w_dram[t, hc * P : (hc + 1) * P, :])
                tiles.append(w_)
        return tiles

    wkvp = tc.alloc_tile_pool(name="wkvp", bufs=1)
    wk_t = load_w(wkvp, wk, "wk")
    wv_t = load_w(wkvp, wv, "wv")

    def load_windows(x_dram, sb):
        win = []
        for hc in range(HC):
            w_ = xwp.tile([P, SBK + 2], bf16, name=f"xw{hc}", tag=f"xw{hc}")
            nc.sync.dma_start(
                w_[:], x_dram[hc * P : (hc + 1) * P, sb * SBK : sb * SBK + SBK + 2]
            )
            win.append(w_)
        return win

    def conv_pair(psum, w_tiles, win, pr, idx):
        t, hc = divmod(idx, HC)
        nc.tensor.matmul(
            psum[:], w_tiles[idx][:, pr * P : (pr + 1) * P], win[hc][:, t : t + SBK],
            start=(idx == 0), stop=(idx == KS * HC - 1),
        )

    with tc.tile_pool(name="psk", bufs=2, space="PSUM") as psk, tc.tile_pool(
        name="psv", bufs=2, space="PSUM"
    ) as psv:
        for sb in range(NSB):
            win = load_windows(xsT, sb)
            for pr in range(NPAIR):
                pk = psk.tile([P, SBK], f32, name="pk", tag="pk")
                for idx in range(KS * HC):
                    conv_pair(pk, wk_t, win, pr, idx)
                nc.vector.tensor_scalar_add(
                    kT[pr][:, sb * SBK : (sb + 1) * SBK], pk[:], bk_sb[:, pr : pr + 1]
                )
            for j in range(SBK // P):
                st = sb * (SBK // P) + j
                pv = psv.tile([P, DL], f32, name="pv", tag="pv")
                for idx in range(KS * HC):
                    t, hc = divmod(idx, HC)
                    nc.tensor.matmul(
                        pv[:], win[hc][:, j * P + t : j * P + t + P], wv_t[idx][:],
                        start=(idx == 0), stop=(idx == KS * HC - 1),
                    )
                vt3 = vt[st][:, 0 : NHL * (D + 1)].rearrange("p (h c) -> p h c", c=D + 1)
                nc.vector.tensor_copy(
                    vt3[:, :, 0:D], pv[:].rearrange("p (h c) -> p h c", c=D)
                )
                nc.sync.dma_start(vt3[:, :, D : D + 1], ones4)
    wkvp.release()

    wqp = tc.alloc_tile_pool(name="wqp", bufs=1)
    wq_t = load_w(wqp, wq, "wq")

    ptp = tc.alloc_tile_pool(name="ptp", bufs=4)
    srecp = tc.alloc_tile_pool(name="srecp", bufs=2)
    aTsp = tc.alloc_tile_pool(name="aTsp", bufs=2)
    osb_p = tc.alloc_tile_pool(name="osb_p", bufs=3)

    ps_s4 = tc.alloc_tile_pool(name="ps_s4", bufs=2, space="PSUM")
    ps_u = tc.alloc_tile_pool(name="ps_u", bufs=1, space="PSUM")
    ps_qe = tc.alloc_tile_pool(name="ps_qe", bufs=1, space="PSUM")
    ps_q = tc.alloc_tile_pool(name="ps_q", bufs=1, space="PSUM")

    dramp = tc.alloc_tile_pool(name="dramp", bufs=1, space="DRAM")
    agin = [dramp.tile([DL, SBK], bf16, name=f"agin{qb}") for qb in range(NSB)]
    agout = [dramp.tile([H, SBK], bf16, name=f"agout{qb}") for qb in range(NSB)]

    for sb in range(NSB):
        win = load_windows(xqT, sb)
        for pr in range(NPAIR):
            pq = ps_q.tile([P, SBK], f32, name="pq", tag="pq")
            for idx in range(KS * HC):
                conv_pair(pq, wq_t, win, pr, idx)
            nc.vector.tensor_scalar_add(
                qT[pr][:, sb * SBK : (sb + 1) * SBK], pq[:], bq_sb[:, pr : pr + 1]
            )
    ps_q.release()
    ps_o = tc.alloc_tile_pool(name="ps_o", bufs=1, space="PSUM")

    def attention(pr, qb):
        qsl = slice(qb * SBK, (qb + 1) * SBK)
        u = [ps_u.tile([D + 1, SBK], f32, name=f"u{h}", tag=f"u{h}") for h in range(2)]
        for kt2 in range(KT // 2):
            kt0, kt1 = 2 * kt2, 2 * kt2 + 1
            s2 = [ps_s4.tile([P, 2 * SBK], f32, name=f"s2{h}", tag="s4") for h in range(2)]
            pt2 = [ptp.tile([P, 2 * SBK], bf16, name=f"pt{h}", tag="pt") for h in range(2)]
            for h, (ha, hb) in enumerate(((0, P // 2), (P // 2, P))):
                hsl = slice(ha, hb)
                for i, kt in enumerate((kt0, kt1)):
                    ksl = slice(kt * P, (kt + 1) * P)
                    nc.tensor.matmul(
                        s2[h][:, i * SBK : (i + 1) * SBK],
                        kT[pr][hsl, ksl],
                        qT[pr][hsl, qsl],
                        start=True, stop=True,
                        tile_position=(ha, 0),
                    )
                if mask_zero:
                    nc.scalar.activation(pt2[h][:], s2[h][:], FT.Exp, scale=SCALE)
                else:
                    for i, kt in enumerate((kt0, kt1)):
                        csl = slice(i * SBK, (i + 1) * SBK)
                        nc.scalar.activation(
                            pt2[h][:, csl], s2[h][:, csl], FT.Exp,
                            bias=maskS[:, kt : kt + 1], scale=SCALE,
                        )
            for i, kt in enumerate((kt0, kt1)):
                for h in range(2):
                    nc.tensor.matmul(
                        u[h][:, :],
                        vt[kt][:, (2 * pr + h) * (D + 1) : (2 * pr + h + 1) * (D + 1)],
                        pt2[h][:, i * SBK : (i + 1) * SBK],
                        start=(kt2 == 0 and i == 0),
                        stop=(kt2 == KT // 2 - 1 and i == 1),
                    )
        for h in range(2):
            hh = 2 * pr + h
            s_sb = srecp.tile([D + 1, SBK], f32, name="s_sb", tag="s_sb")
            nc.vector.tensor_copy(s_sb[D : D + 1, :], u[h][D : D + 1, :])
            srec = srecp.tile([D + 1, SBK], f32r, name="srec", tag="srec")
            with nc.allow_low_precision(reason="softmax 1/s at fp22 is plenty"):
                nc.vector.reciprocal(srec[D : D + 1, :], s_sb[D : D + 1, :])
            bc = ps_qe.tile([D, SBK], f32, name="bc", tag="qe")
            nc.tensor.matmul(
                bc[:, :], ones_row[D : D + 1, 0:D], srec[D : D + 1, :],
                start=True, stop=True, tile_position=(D, 0),
            )
            bc_sb = srecp.tile([D, SBK], f32, name="bc_sb", tag="bc_sb")
            nc.vector.tensor_copy(bc_sb[:], bc[:])
            nc.vector.tensor_mul(aTh[hh][:, qsl], u[h][0:D, :], bc_sb[:])
            nc.vector.tensor_scalar_add(
                aTh[hh][:, qsl], aTh[hh][:, qsl], bv_sb[:, hh : hh + 1]
            )
            nc.sync.dma_start(agin[qb][hh * D : (hh + 1) * D, :], aTh[hh][:, qsl])

    def outproj(qb):
        aTs = []
        for hc in range(HC):
            a_ = aTsp.tile([P, SBK], bf16, name=f"aTs{hc}", tag=f"aTs{hc}")
            nc.sync.dma_start(a_[:], agout[qb][hc * P : (hc + 1) * P, :])
            aTs.append(a_)
        for j in range(SBK // P):
            qt = qb * (SBK // P) + j
            po = ps_o.tile([P, DL], f32, name="po", tag="po")
            for hc in range(HC):
                nc.tensor.matmul(
                    po[:],
                    aTs[hc][:, j * P : (j + 1) * P],
                    wo_sb[:, hc * DL : (hc + 1) * DL],
                    start=(hc == 0), stop=(hc == HC - 1),
                )
            osb = osb_p.tile([P, DL], f32, name="osb", tag="osb")
            nc.vector.tensor_add(osb[:], po[:], bo_sb[:])
            nc.sync.dma_start(out[qt * P : (qt + 1) * P, :], osb[:])

    for qb in range(NSB):
        for pr in range(NPAIR):
            attention(pr, qb)
        nc.gpsimd.collective_compute(
            "AllGather",
            mybir.AluOpType.bypass,
            replica_groups=REPLICA_GROUPS,
            ins=[agin[qb].opt()],
            outs=[agout[qb].opt()],
        )
        outproj(qb)

    for pool in (
        ps_o, ps_qe, ps_u, ps_s4, osb_p, aTsp, srecp, ptp, wqp, xwp,
    ):
        pool.release()
    dramp.release()
    bigp.release()
    constp.release()


_BUILD_CACHE = {}


def get_build(mask_zero: bool):
    if mask_zero not in _BUILD_CACHE:
        _BUILD_CACHE[mask_zero] = build(mask_zero)
    return _BUILD_CACHE[mask_zero]


def shard_inputs(inputs):
    qi = np.ascontiguousarray(np.asarray(inputs["query_input"], dtype=np.float32))
    si = np.ascontiguousarray(np.asarray(inputs["source_input"], dtype=np.float32))
    mask = np.asarray(inputs["mask"], dtype=np.float32)
    Wq = np.asarray(inputs["Wq"], dtype=np.float32)
    Wk = np.asarray(inputs["Wk"], dtype=np.float32)
    Wv = np.asarray(inputs["Wv"], dtype=np.float32)
    Wo = np.asarray(inputs["Wo"], dtype=np.float32)
    bq_ = np.asarray(inputs["bq"], dtype=np.float32)
    bk_ = np.asarray(inputs["bk"], dtype=np.float32)
    bv_ = np.asarray(inputs["bv"], dtype=np.float32)
    bo_ = np.asarray(inputs["bo"], dtype=np.float32)

    bf = ml_dtypes.bfloat16
    xT = []
    for b in range(B):
        for x in (qi, si):
            p = np.zeros((H, S + 2), dtype=bf)
            p[:, 1 : S + 1] = x[b].T.astype(bf)
            xT.append(np.ascontiguousarray(p))
    in_maps = []
    for c in range(NCORES):
        b, g = divmod(c, GPB)
        cols = slice(g * DL, (g + 1) * DL)
        in_maps.append(
            dict(
                xqT=xT[2 * b],
                xsT=xT[2 * b + 1],
                wq=np.ascontiguousarray(Wq[:, :, cols].astype(bf)),
                wk=np.ascontiguousarray(Wk[:, :, cols].astype(bf)),
                wv=np.ascontiguousarray(Wv[:, :, cols].astype(bf)),
                wo=np.ascontiguousarray(Wo[:, cols].astype(bf)),
                bq=np.ascontiguousarray(bq_[cols].reshape(NPAIR, P, 1)),
                bk=np.ascontiguousarray(bk_[cols].reshape(NPAIR, P, 1)),
                bv=np.ascontiguousarray(bv_[cols].reshape(NHL, D).T),
                bo_b=np.ascontiguousarray(
                    np.broadcast_to(bo_[cols], (P, DL)).astype(np.float32)
                ),
                maskT=np.ascontiguousarray(mask[b, 0, 0].reshape(KT, P).T),
                ones4=np.ones((P, NHL), dtype=bf),
                ones_r=np.ones((1, P), dtype=np.float32),
            )
        )
    return in_maps, bool(not np.any(mask))


def unshard(core_outs):
    full = np.empty((B, S, H), dtype=np.float32)
    for c in range(NCORES):
        b, g = divmod(c, GPB)
        full[b, :, g * DL : (g + 1) * DL] = core_outs[c]["out"]
    return full


def run(inputs, **kwargs):
    in_maps, mask_zero = shard_inputs(inputs)
    nc = get_build(mask_zero)
    res = bass_utils.run_bass_kernel_spmd(
        nc, in_maps, core_ids=list(range(NCORES)), **kwargs
    )
    return unshard(res.results), res


def kernel(**inputs):
    out, _ = run(inputs)
    return out
